# revision 16
# baseline (speedup 1.0000x reference)
"""GCN encoder (2-layer GCNConv) as a Bass/Tile kernel on 8 Trainium2 NeuronCores.

Strategy (matches the sharding hint):
  - Nodes row-partitioned across 8 cores (6250 rows each); weights replicated.
  - Symmetric normalization factorized: z = D^-1/2 (A+I) D^-1/2 (x W) + b
    =>  u = dinv * (x W);  agg[d] = u[d] + sum_{e:dst=d} u[src_e];
        z = dinv * agg + b
    so no per-edge norm gather is needed.
  - Per layer: local matmul -> row scale -> AllGather(u) -> per-core gather of
    source rows (dma_gather) -> segment-sum via tensor-engine matmuls with
    compile-time-structured 0/1 selection matrices generated on DVE
    (is_equal against an iota) -> scale/bias/relu -> output rows.
  - Edges are bucketed host-side by (dst window of 128, src half) and padded to
    128-slot tiles; padded slots gather row 0 and have an all-zero selection
    column, so they contribute nothing.  int16 gather indices require the
    src-half split (indices < 32768).
"""

import math
import os
import sys

import numpy as np

sys.path.insert(0, "/opt/trn_rl_repo")

import ml_dtypes

BF16 = ml_dtypes.bfloat16


class Cfg:
    def __init__(self, N, E, IN=512, HID=256, OUT=128, P=8, half=None):
        self.N, self.E, self.IN, self.HID, self.OUT, self.P = N, E, IN, HID, OUT, P
        self.NC = N // P                      # nodes per core
        self.WS = 128                         # dst window size
        self.NW = math.ceil(self.NC / self.WS)  # windows per core
        # src-half split point (int16 gather indices must stay < 32768)
        if half is None:
            half = N if N <= 32767 else (N + 1) // 2
        self.HALF = half
        assert self.HALF <= 32767 and N - self.HALF <= 32767


FULL = Cfg(N=50000, E=800000)


def _prepare(cfg, x, edge_index, W1, b1, W2, b2):
    """Host-side graph preprocessing -> per-core input maps + program params."""
    N, P, NC, WS, NW, HALF = cfg.N, cfg.P, cfg.NC, cfg.WS, cfg.NW, cfg.HALF
    src = np.asarray(edge_index[0], dtype=np.int64)
    dst = np.asarray(edge_index[1], dtype=np.int64)

    deg = np.bincount(dst, minlength=N).astype(np.float64) + 1.0  # + self loop
    dinv = (1.0 / np.sqrt(deg)).astype(np.float32)

    # group id: ((core, window), src-half) ; groups contiguous after sort
    win_id = (dst // NC) * NW + (dst % NC) // WS
    half = (src >= HALF).astype(np.int64)
    comp = win_id * 2 + half
    order = np.argsort(comp, kind="stable")
    s_s, d_s, c_s = src[order], dst[order], comp[order]
    counts = np.bincount(c_s, minlength=P * NW * 2).reshape(P, NW, 2)

    # shared tile counts per (window, half): max over cores
    T = np.ceil(counts.max(axis=0) / 128).astype(np.int64)  # [NW, 2]
    tiles_total = int(T.sum())
    slots_total = tiles_total * 128

    starts = np.zeros(P * NW * 2 + 1, dtype=np.int64)
    np.cumsum(counts.reshape(-1), out=starts[1:])

    dinv_pad = np.concatenate(
        [dinv, np.ones(NW * WS * P - N, dtype=np.float32)])

    in_maps = []
    for c in range(P):
        idx_arr = np.zeros(slots_total, dtype=np.int16)
        aco_arr = np.full(slots_total, -1, dtype=np.float32)
        off = 0
        for w in range(NW):
            for h in range(2):
                g = (c * NW + w) * 2 + h
                n = counts[c, w, h]
                sl = slice(starts[g], starts[g] + n)
                idx_arr[off:off + n] = (s_s[sl] - h * HALF).astype(np.int16)
                aco_arr[off:off + n] = (d_s[sl] - c * NC - w * WS).astype(np.float32)
                off += 128 * int(T[w, h])
        assert off == slots_total

        dloc = np.concatenate(
            [dinv[c * NC:(c + 1) * NC],
             np.ones(NW * WS - NC, dtype=np.float32)])

        m = {
            "xT": np.ascontiguousarray(
                np.asarray(x[c * NC:(c + 1) * NC], np.float32).astype(BF16).T),
            "w1": np.ascontiguousarray(
                np.asarray(W1, np.float32).astype(BF16)
                .reshape(cfg.IN // 128, 128, cfg.HID).transpose(1, 0, 2)),
            "w2": np.ascontiguousarray(
                np.asarray(W2, np.float32).astype(BF16)
                .reshape(cfg.HID // 128, 128, cfg.OUT).transpose(1, 0, 2)),
            "dinvc": np.ascontiguousarray(dloc.reshape(NW, WS).T),
            "idx": np.ascontiguousarray(np.tile(idx_arr.reshape(-1, 16).T, (8, 1))),
            "acol": np.ascontiguousarray(aco_arr.reshape(-1, 128).T),
            "ident": np.eye(128, dtype=BF16),
        }
        b1nz = bool(np.any(np.asarray(b1)))
        b2nz = bool(np.any(np.asarray(b2)))
        if b1nz:
            m["b1bc"] = np.ascontiguousarray(
                np.broadcast_to(np.asarray(b1, np.float32), (128, cfg.HID)))
        if b2nz:
            m["b2bc"] = np.ascontiguousarray(
                np.broadcast_to(np.asarray(b2, np.float32), (128, cfg.OUT)))
        in_maps.append(m)

    return in_maps, T, b1nz, b2nz


def build_program(cfg, T, b1nz, b2nz):
    import concourse.bass as bass
    import concourse.bacc as bacc
    import concourse.mybir as mybir
    from concourse import tile

    N, P, NC, WS, NW = cfg.N, cfg.P, cfg.NC, cfg.WS, cfg.NW
    IN, HID, OUT = cfg.IN, cfg.HID, cfg.OUT
    NCI, NCH = IN // 128, HID // 128
    tiles_total = int(T.sum())
    slots_total = tiles_total * 128
    f32, bf16, i16 = mybir.dt.float32, mybir.dt.bfloat16, mybir.dt.int16
    AF = mybir.ActivationFunctionType

    nc = bacc.Bacc("TRN2", target_bir_lowering=False, debug=False,
                   num_devices=cfg.P)
    xT_p = nc.dram_tensor("xT", [IN, NC], bf16, kind="ExternalInput")
    w1_p = nc.dram_tensor("w1", [128, NCI, HID], bf16, kind="ExternalInput")
    w2_p = nc.dram_tensor("w2", [128, NCH, OUT], bf16, kind="ExternalInput")
    dinv_p = nc.dram_tensor("dinvc", [WS, NW], f32, kind="ExternalInput")
    idx_p = nc.dram_tensor("idx", [128, slots_total // 16], i16, kind="ExternalInput")
    acol_p = nc.dram_tensor("acol", [128, tiles_total], f32, kind="ExternalInput")
    id_p = nc.dram_tensor("ident", [128, 128], bf16, kind="ExternalInput")
    b1_p = (nc.dram_tensor("b1bc", [128, HID], f32, kind="ExternalInput")
            if b1nz else None)
    b2_p = (nc.dram_tensor("b2bc", [128, OUT], f32, kind="ExternalInput")
            if b2nz else None)
    out_p = nc.dram_tensor("out", [NC, OUT], f32, kind="ExternalOutput")

    u1d = nc.dram_tensor("u1d", [NC, HID], bf16)
    u2d = nc.dram_tensor("u2d", [NC, OUT], bf16)
    U1 = nc.dram_tensor("U1", [N, HID], bf16)
    U2 = nc.dram_tensor("U2", [N, OUT], bf16)
    rg = [list(range(P))]

    with tile.TileContext(nc) as tc:
        with (
            tc.tile_pool(name="res", bufs=1) as res,
            tc.tile_pool(name="work", bufs=4) as work,
            tc.tile_pool(name="gath", bufs=4) as gath,
            tc.tile_pool(name="psum", bufs=2, space="PSUM") as psum,
        ):
            # ---- resident loads ----
            xTs = res.tile([128, NCI, NC], bf16)
            for ci in range(NCI):
                nc.sync.dma_start(xTs[:, ci, :], xT_p[ci * 128:(ci + 1) * 128, :])
            w1s = res.tile([128, NCI, HID], bf16)
            nc.sync.dma_start(w1s[:], w1_p[:])
            w2s = res.tile([128, NCH, OUT], bf16)
            nc.sync.dma_start(w2s[:], w2_p[:])
            dinvs = res.tile([WS, NW], f32)
            nc.sync.dma_start(dinvs[:], dinv_p[:])
            idxs = res.tile([128, slots_total // 16], i16)
            nc.sync.dma_start(idxs[:], idx_p[:])
            acols = res.tile([128, tiles_total], f32)
            nc.sync.dma_start(acols[:], acol_p[:])
            ident = res.tile([128, 128], bf16)
            nc.sync.dma_start(ident[:], id_p[:])
            iot = res.tile([128, 128], f32)
            nc.gpsimd.iota(iot[:], pattern=[[1, 128]], base=0,
                           channel_multiplier=0,
                           allow_small_or_imprecise_dtypes=True)
            b1bc = None
            if b1nz:
                b1bc = res.tile([128, HID], f32)
                nc.sync.dma_start(b1bc[:], b1_p[:])
            b2bc = None
            if b2nz:
                b2bc = res.tile([128, OUT], f32)
                nc.sync.dma_start(b2bc[:], b2_p[:])

            u1res = res.tile([128, NW, HID], bf16)
            u2res = res.tile([128, NW, OUT], bf16)
            h1T = res.tile([128, NCH, NC], bf16)
            if NC % WS:
                # tail rows of the last window feed the self-loop matmul as
                # rhs; zero them so uninitialized SBUF can't inject NaNs
                nc.gpsimd.memset(u1res[:, NW - 1, :], 0.0)
                nc.gpsimd.memset(u2res[:, NW - 1, :], 0.0)

            def nsz(j):
                return min(128, NC - j * WS)

            MAXP = int(os.environ.get("GCN_MAX_PHASE", "9"))

            def emit_debug_out(src_bf16_ap, w, n):
                # convert [n, OUT] bf16 -> f32, dump into out rows of window w
                dt = work.tile([128, OUT], f32, tag="dbg")
                nc.scalar.activation(dt[:n, :], src_bf16_ap, AF.Copy)
                nc.sync.dma_start(out_p[w * WS:w * WS + n, :], dt[:n, :])

            # ---- phase A: t1 = x @ W1 ; u1 = dinv * t1 ----
            for j in range(NW):
                n = nsz(j)
                jsl = slice(j * WS, j * WS + n)
                pt = psum.tile([128, HID], f32, tag="mm")
                for ci in range(NCI):
                    nc.tensor.matmul(pt[:n, :], xTs[:, ci, jsl],
                                     w1s[:, ci, :], start=(ci == 0),
                                     stop=(ci == NCI - 1))
                nc.scalar.activation(u1res[:n, j, :], pt[:n, :], AF.Copy,
                                     scale=dinvs[:n, j:j + 1])
                nc.sync.dma_start(u1d[jsl, :], u1res[:n, j, :])
                if MAXP == 1:
                    emit_debug_out(u1res[:n, j, :OUT], j, n)
            if MAXP <= 1:
                return nc

            # ---- AllGather u1 ----
            nc.gpsimd.collective_compute(
                "AllGather", mybir.AluOpType.bypass, replica_groups=rg,
                ins=[u1d[:]], outs=[U1[:]])
            if MAXP == 2:
                for j in range(NW):
                    n = nsz(j)
                    gt = work.tile([128, OUT], bf16, tag="dbg_g")
                    nc.sync.dma_start(gt[:n, :], U1[j * WS:j * WS + n, :OUT])
                    emit_debug_out(gt[:n, :], j, n)
                return nc

            # ---- generic aggregation layer ----
            def agg_layer(U, F, ures, bbc, relu, emit_out):
                tile_idx = 0
                slot_off = 0
                for w in range(NW):
                    n = nsz(w)
                    pa = psum.tile([128, F], f32, tag="agg")
                    # self-loop term: ident.T @ u[w]
                    nc.tensor.matmul(pa[:n, :], ident[:, :n], ures[:, w, :],
                                     start=True, stop=False)
                    nmm = int(T[w, 0] + T[w, 1])
                    done = 0
                    for h in range(2):
                        t_wh = int(T[w, h])
                        if t_wh == 0:
                            continue
                        g = gath.tile([128, t_wh, F], bf16, tag="g")
                        base = 0 if h == 0 else cfg.HALF
                        nc.gpsimd.dma_gather(
                            g[:], U[base:base + min(cfg.HALF, N - base), :],
                            idxs[:, slot_off // 16:
                                 (slot_off + 128 * t_wh) // 16],
                            num_idxs=128 * t_wh, num_idxs_reg=128 * t_wh,
                            elem_size=F, single_packet=False)
                        slot_off += 128 * t_wh
                        for t in range(t_wh):
                            S = work.tile([128, 128], bf16, tag="S")
                            seng = (nc.vector if os.environ.get(
                                "GCN_SGEN_ENGINE", "gpsimd") == "vector"
                                else nc.gpsimd)
                            seng.tensor_scalar(
                                S[:], iot[:],
                                acols[:, tile_idx:tile_idx + 1], None,
                                op0=mybir.AluOpType.is_equal)
                            tile_idx += 1
                            done += 1
                            nc.tensor.matmul(pa[:n, :], S[:, :n], g[:, t, :],
                                             start=False, stop=(done == nmm))
                    # z = dinv * agg (+ b) ; relu
                    if bbc is None:
                        zf = AF.Relu if relu else AF.Copy
                        zt = work.tile([128, F], f32 if emit_out else bf16,
                                       tag="zt%d" % F)
                        nc.scalar.activation(zt[:n, :], pa[:n, :], zf,
                                             scale=dinvs[:n, w:w + 1])
                    else:
                        v = work.tile([128, F], f32, tag="v%d" % F)
                        nc.scalar.activation(v[:n, :], pa[:n, :], AF.Copy,
                                             scale=dinvs[:n, w:w + 1])
                        zt = work.tile([128, F], f32 if emit_out else bf16,
                                       tag="zt%d" % F)
                        if relu:
                            vb = work.tile([128, F], f32, tag="vb%d" % F)
                            nc.vector.tensor_tensor(
                                vb[:n, :], v[:n, :], bbc[:n, :],
                                op=mybir.AluOpType.add)
                            nc.scalar.activation(zt[:n, :], vb[:n, :], AF.Relu)
                        else:
                            nc.vector.tensor_tensor(
                                zt[:n, :], v[:n, :], bbc[:n, :],
                                op=mybir.AluOpType.add)
                    yield w, n, zt

            # ---- phase C: layer-1 aggregation -> h1 -> h1T ----
            for w, n, zt in agg_layer(U1, HID, u1res, b1bc, True, False):
                wsl = slice(w * WS, w * WS + n)
                for ch in range(NCH):
                    ptr = psum.tile([128, 128], bf16, tag="tr")
                    nc.tensor.transpose(ptr[:, :n],
                                        zt[:n, ch * 128:(ch + 1) * 128],
                                        ident[:n, :n])
                    nc.scalar.activation(h1T[:, ch, wsl], ptr[:, :n], AF.Copy)
                if MAXP == 3:
                    emit_debug_out(zt[:n, :OUT], w, n)
            if MAXP <= 3:
                return nc

            # ---- phase D: t2 = h1 @ W2 ; u2 ----
            for j in range(NW):
                n = nsz(j)
                jsl = slice(j * WS, j * WS + n)
                pt = psum.tile([128, OUT], f32, tag="mm")
                for ch in range(NCH):
                    nc.tensor.matmul(pt[:n, :], h1T[:, ch, jsl],
                                     w2s[:, ch, :], start=(ch == 0),
                                     stop=(ch == NCH - 1))
                nc.scalar.activation(u2res[:n, j, :], pt[:n, :], AF.Copy,
                                     scale=dinvs[:n, j:j + 1])
                nc.sync.dma_start(u2d[jsl, :], u2res[:n, j, :])
                if MAXP == 4:
                    emit_debug_out(u2res[:n, j, :], j, n)
            if MAXP <= 4:
                return nc

            # ---- AllGather u2 ----
            nc.gpsimd.collective_compute(
                "AllGather", mybir.AluOpType.bypass, replica_groups=rg,
                ins=[u2d[:]], outs=[U2[:]])

            # ---- phase F: layer-2 aggregation -> out ----
            for w, n, zt in agg_layer(U2, OUT, u2res, b2bc, False, True):
                wsl = slice(w * WS, w * WS + n)
                nc.sync.dma_start(out_p[wsl, :], zt[:n, :])

    return nc


def run(cfg, inputs, sim=False, trace=False):
    from concourse.bass_utils import run_bass_kernel_spmd

    in_maps, T, b1nz, b2nz = _prepare(
        cfg, inputs["x"], inputs["edge_index"], inputs["W1"], inputs["b1"],
        inputs["W2"], inputs["b2"])
    nc = build_program(cfg, T, b1nz, b2nz)
    nc.finalize()
    core_ids = list(range(cfg.P))
    if sim:
        from concourse import bass_interp
        ms = bass_interp.MultiCoreSim(nc, cfg.P)
        for c in core_ids:
            for k, v in in_maps[c].items():
                ms.cores[c].tensor(k)[:] = v
        ms.simulate()
        outs = [np.array(ms.cores[c].tensor("out")) for c in core_ids]
        return np.concatenate(outs, axis=0), None
    res = run_bass_kernel_spmd(nc, in_maps, core_ids, trace=trace)
    outs = [np.asarray(res.results[c]["out"]) for c in core_ids]
    return np.concatenate(outs, axis=0), res


def kernel(x, edge_index, W1, b1, W2, b2):
    out, _ = run(FULL, dict(x=x, edge_index=edge_index, W1=W1, b1=b1,
                            W2=W2, b2=b2))
    return out


# revision 17
# speedup vs baseline: 2.6012x; 2.6012x over previous
"""GCN encoder (2-layer GCNConv) as a Bass/Tile kernel on 8 Trainium2 NeuronCores.

Strategy (matches the sharding hint):
  - Nodes row-partitioned across 8 cores (6250 rows each); weights replicated.
  - Symmetric normalization factorized: z = D^-1/2 (A+I) D^-1/2 (x W) + b
    =>  u = dinv * (x W);  agg[d] = u[d] + sum_{e:dst=d} u[src_e];
        z = dinv * agg + b
    so no per-edge norm gather is needed.
  - Per layer: local matmul -> row scale -> AllGather(u) -> per-core gather of
    source rows (dma_gather) -> segment-sum via tensor-engine matmuls with
    compile-time-structured 0/1 selection matrices generated on DVE
    (is_equal against an iota) -> scale/bias/relu -> output rows.
  - Edges are bucketed host-side by (dst window of 128, src half) and padded to
    128-slot tiles; padded slots gather row 0 and have an all-zero selection
    column, so they contribute nothing.  int16 gather indices require the
    src-half split (indices < 32768).
"""

import math
import os
import sys

import numpy as np

sys.path.insert(0, "/opt/trn_rl_repo")

import ml_dtypes

BF16 = ml_dtypes.bfloat16


class Cfg:
    def __init__(self, N, E, IN=512, HID=256, OUT=128, P=8, half=None):
        self.N, self.E, self.IN, self.HID, self.OUT, self.P = N, E, IN, HID, OUT, P
        self.NC = N // P                      # nodes per core
        self.WS = 128                         # dst window size
        self.NW = math.ceil(self.NC / self.WS)  # windows per core
        # src-half split point (int16 gather indices must stay < 32768)
        if half is None:
            half = N if N <= 32767 else (N + 1) // 2
        self.HALF = half
        assert self.HALF <= 32767 and N - self.HALF <= 32767


FULL = Cfg(N=50000, E=800000)


def _prepare(cfg, x, edge_index, W1, b1, W2, b2):
    """Host-side graph preprocessing -> per-core input maps + program params."""
    N, P, NC, WS, NW, HALF = cfg.N, cfg.P, cfg.NC, cfg.WS, cfg.NW, cfg.HALF
    src = np.asarray(edge_index[0], dtype=np.int64)
    dst = np.asarray(edge_index[1], dtype=np.int64)

    deg = np.bincount(dst, minlength=N).astype(np.float64) + 1.0  # + self loop
    dinv = (1.0 / np.sqrt(deg)).astype(np.float32)

    # group id: ((core, window), src-half) ; groups contiguous after sort
    win_id = (dst // NC) * NW + (dst % NC) // WS
    half = (src >= HALF).astype(np.int64)
    comp = win_id * 2 + half
    order = np.argsort(comp, kind="stable")
    s_s, d_s, c_s = src[order], dst[order], comp[order]
    counts = np.bincount(c_s, minlength=P * NW * 2).reshape(P, NW, 2)

    # shared tile counts per (window, half): max over cores
    T = np.ceil(counts.max(axis=0) / 128).astype(np.int64)  # [NW, 2]
    tiles_total = int(T.sum())
    slots_total = tiles_total * 128

    starts = np.zeros(P * NW * 2 + 1, dtype=np.int64)
    np.cumsum(counts.reshape(-1), out=starts[1:])

    dinv_pad = np.concatenate(
        [dinv, np.ones(NW * WS * P - N, dtype=np.float32)])

    in_maps = []
    for c in range(P):
        idx_arr = np.zeros(slots_total, dtype=np.int16)
        aco_arr = np.full(slots_total, -1, dtype=np.float32)
        off = 0
        for w in range(NW):
            for h in range(2):
                g = (c * NW + w) * 2 + h
                n = counts[c, w, h]
                sl = slice(starts[g], starts[g] + n)
                idx_arr[off:off + n] = (s_s[sl] - h * HALF).astype(np.int16)
                aco_arr[off:off + n] = (d_s[sl] - c * NC - w * WS).astype(np.float32)
                off += 128 * int(T[w, h])
        assert off == slots_total

        dloc = np.concatenate(
            [dinv[c * NC:(c + 1) * NC],
             np.ones(NW * WS - NC, dtype=np.float32)])

        m = {
            "xT": np.ascontiguousarray(
                np.asarray(x[c * NC:(c + 1) * NC], np.float32).astype(BF16).T),
            "w1": np.ascontiguousarray(
                np.asarray(W1, np.float32).astype(BF16)
                .reshape(cfg.IN // 128, 128, cfg.HID).transpose(1, 0, 2)),
            "w2": np.ascontiguousarray(
                np.asarray(W2, np.float32).astype(BF16)
                .reshape(cfg.HID // 128, 128, cfg.OUT).transpose(1, 0, 2)),
            "dinvc": np.ascontiguousarray(dloc.reshape(NW, WS).T),
            "idx": np.ascontiguousarray(np.tile(idx_arr.reshape(-1, 16).T, (8, 1))),
            "acol": np.ascontiguousarray(aco_arr.reshape(-1, 128).T),
            "ident": np.eye(128, dtype=BF16),
        }
        b1nz = bool(np.any(np.asarray(b1)))
        b2nz = bool(np.any(np.asarray(b2)))
        if b1nz:
            m["b1bc"] = np.ascontiguousarray(
                np.broadcast_to(np.asarray(b1, np.float32), (128, cfg.HID)))
        if b2nz:
            m["b2bc"] = np.ascontiguousarray(
                np.broadcast_to(np.asarray(b2, np.float32), (128, cfg.OUT)))
        in_maps.append(m)

    return in_maps, T, b1nz, b2nz


def build_program(cfg, T, b1nz, b2nz):
    import concourse.bass as bass
    import concourse.bacc as bacc
    import concourse.mybir as mybir
    from concourse import tile

    N, P, NC, WS, NW = cfg.N, cfg.P, cfg.NC, cfg.WS, cfg.NW
    IN, HID, OUT = cfg.IN, cfg.HID, cfg.OUT
    NCI, NCH = IN // 128, HID // 128
    tiles_total = int(T.sum())
    slots_total = tiles_total * 128
    f32, bf16, i16 = mybir.dt.float32, mybir.dt.bfloat16, mybir.dt.int16
    AF = mybir.ActivationFunctionType

    nc = bacc.Bacc("TRN2", target_bir_lowering=False, debug=False,
                   num_devices=cfg.P)
    xT_p = nc.dram_tensor("xT", [IN, NC], bf16, kind="ExternalInput")
    w1_p = nc.dram_tensor("w1", [128, NCI, HID], bf16, kind="ExternalInput")
    w2_p = nc.dram_tensor("w2", [128, NCH, OUT], bf16, kind="ExternalInput")
    dinv_p = nc.dram_tensor("dinvc", [WS, NW], f32, kind="ExternalInput")
    idx_p = nc.dram_tensor("idx", [128, slots_total // 16], i16, kind="ExternalInput")
    acol_p = nc.dram_tensor("acol", [128, tiles_total], f32, kind="ExternalInput")
    id_p = nc.dram_tensor("ident", [128, 128], bf16, kind="ExternalInput")
    b1_p = (nc.dram_tensor("b1bc", [128, HID], f32, kind="ExternalInput")
            if b1nz else None)
    b2_p = (nc.dram_tensor("b2bc", [128, OUT], f32, kind="ExternalInput")
            if b2nz else None)
    out_p = nc.dram_tensor("out", [NC, OUT], f32, kind="ExternalOutput")

    u1d = nc.dram_tensor("u1d", [NC, HID], bf16)
    u2d = nc.dram_tensor("u2d", [NC, OUT], bf16)
    U1 = nc.dram_tensor("U1", [N, HID], bf16)
    U2 = nc.dram_tensor("U2", [N, OUT], bf16)
    rg = [list(range(P))]

    with tile.TileContext(nc) as tc:
        with (
            tc.tile_pool(name="res", bufs=1) as res,
            tc.tile_pool(name="work", bufs=4) as work,
            tc.tile_pool(name="gath", bufs=4) as gath,
            tc.tile_pool(name="psum", bufs=2, space="PSUM") as psum,
        ):
            # ---- resident loads ----
            xTs = res.tile([128, NCI, NC], bf16)
            for ci in range(NCI):
                nc.sync.dma_start(xTs[:, ci, :], xT_p[ci * 128:(ci + 1) * 128, :])
            w1s = res.tile([128, NCI, HID], bf16)
            nc.sync.dma_start(w1s[:], w1_p[:])
            w2s = res.tile([128, NCH, OUT], bf16)
            nc.sync.dma_start(w2s[:], w2_p[:])
            dinvs = res.tile([WS, NW], f32)
            nc.sync.dma_start(dinvs[:], dinv_p[:])
            idxs = res.tile([128, slots_total // 16], i16)
            nc.sync.dma_start(idxs[:], idx_p[:])
            acols = res.tile([128, tiles_total], f32)
            nc.sync.dma_start(acols[:], acol_p[:])
            ident = res.tile([128, 128], bf16)
            nc.sync.dma_start(ident[:], id_p[:])
            iot = res.tile([128, 128], f32)
            nc.gpsimd.iota(iot[:], pattern=[[1, 128]], base=0,
                           channel_multiplier=0,
                           allow_small_or_imprecise_dtypes=True)
            b1bc = None
            if b1nz:
                b1bc = res.tile([128, HID], f32)
                nc.sync.dma_start(b1bc[:], b1_p[:])
            b2bc = None
            if b2nz:
                b2bc = res.tile([128, OUT], f32)
                nc.sync.dma_start(b2bc[:], b2_p[:])

            u1res = res.tile([128, NW, HID], bf16)
            u2res = res.tile([128, NW, OUT], bf16)
            h1T = res.tile([128, NCH, NC], bf16)
            if NC % WS:
                # tail rows of the last window feed the self-loop matmul as
                # rhs; zero them so uninitialized SBUF can't inject NaNs
                nc.gpsimd.memset(u1res[:, NW - 1, :], 0.0)
                nc.gpsimd.memset(u2res[:, NW - 1, :], 0.0)

            def nsz(j):
                return min(128, NC - j * WS)

            MAXP = int(os.environ.get("GCN_MAX_PHASE", "9"))

            def emit_debug_out(src_bf16_ap, w, n):
                # convert [n, OUT] bf16 -> f32, dump into out rows of window w
                dt = work.tile([128, OUT], f32, tag="dbg")
                nc.scalar.activation(dt[:n, :], src_bf16_ap, AF.Copy)
                nc.sync.dma_start(out_p[w * WS:w * WS + n, :], dt[:n, :])

            # ---- phase A: t1 = x @ W1 ; u1 = dinv * t1 ----
            for j in range(NW):
                n = nsz(j)
                jsl = slice(j * WS, j * WS + n)
                pt = psum.tile([128, HID], f32, tag="mm")
                for ci in range(NCI):
                    nc.tensor.matmul(pt[:n, :], xTs[:, ci, jsl],
                                     w1s[:, ci, :], start=(ci == 0),
                                     stop=(ci == NCI - 1))
                nc.scalar.activation(u1res[:n, j, :], pt[:n, :], AF.Copy,
                                     scale=dinvs[:n, j:j + 1])
                nc.sync.dma_start(u1d[jsl, :], u1res[:n, j, :])
                if MAXP == 1:
                    emit_debug_out(u1res[:n, j, :OUT], j, n)
            if MAXP <= 1:
                return nc

            # ---- AllGather u1 ----
            nc.gpsimd.collective_compute(
                "AllGather", mybir.AluOpType.bypass, replica_groups=rg,
                ins=[u1d[:]], outs=[U1[:]])
            if MAXP == 2:
                for j in range(NW):
                    n = nsz(j)
                    gt = work.tile([128, OUT], bf16, tag="dbg_g")
                    nc.sync.dma_start(gt[:n, :], U1[j * WS:j * WS + n, :OUT])
                    emit_debug_out(gt[:n, :], j, n)
                return nc

            # ---- generic aggregation layer ----
            def agg_layer(U, F, ures, bbc, relu, emit_out):
                tile_idx = 0
                slot_off = 0
                for w in range(NW):
                    n = nsz(w)
                    pa = psum.tile([128, F], f32, tag="agg")
                    # self-loop term: ident.T @ u[w]
                    nc.tensor.matmul(pa[:n, :], ident[:, :n], ures[:, w, :],
                                     start=True, stop=False)
                    nmm = int(T[w, 0] + T[w, 1])
                    done = 0
                    for h in range(2):
                        t_wh = int(T[w, h])
                        if t_wh == 0:
                            continue
                        g = gath.tile([128, t_wh, F], bf16, tag="g")
                        base = 0 if h == 0 else cfg.HALF
                        nc.gpsimd.dma_gather(
                            g[:], U[base:base + min(cfg.HALF, N - base), :],
                            idxs[:, slot_off // 16:
                                 (slot_off + 128 * t_wh) // 16],
                            num_idxs=128 * t_wh, num_idxs_reg=128 * t_wh,
                            elem_size=F, single_packet=False)
                        slot_off += 128 * t_wh
                        for t in range(t_wh):
                            S = work.tile([128, 128], bf16, tag="S")
                            seng = (nc.vector if os.environ.get(
                                "GCN_SGEN_ENGINE", "vector") == "vector"
                                else nc.gpsimd)
                            seng.tensor_scalar(
                                S[:], iot[:],
                                acols[:, tile_idx:tile_idx + 1], None,
                                op0=mybir.AluOpType.is_equal)
                            tile_idx += 1
                            done += 1
                            nc.tensor.matmul(pa[:n, :], S[:, :n], g[:, t, :],
                                             start=False, stop=(done == nmm))
                    # z = dinv * agg (+ b) ; relu
                    if bbc is None:
                        zf = AF.Relu if relu else AF.Copy
                        zt = work.tile([128, F], f32 if emit_out else bf16,
                                       tag="zt%d" % F)
                        nc.scalar.activation(zt[:n, :], pa[:n, :], zf,
                                             scale=dinvs[:n, w:w + 1])
                    else:
                        v = work.tile([128, F], f32, tag="v%d" % F)
                        nc.scalar.activation(v[:n, :], pa[:n, :], AF.Copy,
                                             scale=dinvs[:n, w:w + 1])
                        zt = work.tile([128, F], f32 if emit_out else bf16,
                                       tag="zt%d" % F)
                        if relu:
                            vb = work.tile([128, F], f32, tag="vb%d" % F)
                            nc.vector.tensor_tensor(
                                vb[:n, :], v[:n, :], bbc[:n, :],
                                op=mybir.AluOpType.add)
                            nc.scalar.activation(zt[:n, :], vb[:n, :], AF.Relu)
                        else:
                            nc.vector.tensor_tensor(
                                zt[:n, :], v[:n, :], bbc[:n, :],
                                op=mybir.AluOpType.add)
                    yield w, n, zt

            # ---- phase C: layer-1 aggregation -> h1 -> h1T ----
            for w, n, zt in agg_layer(U1, HID, u1res, b1bc, True, False):
                wsl = slice(w * WS, w * WS + n)
                for ch in range(NCH):
                    ptr = psum.tile([128, 128], bf16, tag="tr")
                    nc.tensor.transpose(ptr[:, :n],
                                        zt[:n, ch * 128:(ch + 1) * 128],
                                        ident[:n, :n])
                    nc.scalar.activation(h1T[:, ch, wsl], ptr[:, :n], AF.Copy)
                if MAXP == 3:
                    emit_debug_out(zt[:n, :OUT], w, n)
            if MAXP <= 3:
                return nc

            # ---- phase D: t2 = h1 @ W2 ; u2 ----
            for j in range(NW):
                n = nsz(j)
                jsl = slice(j * WS, j * WS + n)
                pt = psum.tile([128, OUT], f32, tag="mm")
                for ch in range(NCH):
                    nc.tensor.matmul(pt[:n, :], h1T[:, ch, jsl],
                                     w2s[:, ch, :], start=(ch == 0),
                                     stop=(ch == NCH - 1))
                nc.scalar.activation(u2res[:n, j, :], pt[:n, :], AF.Copy,
                                     scale=dinvs[:n, j:j + 1])
                nc.sync.dma_start(u2d[jsl, :], u2res[:n, j, :])
                if MAXP == 4:
                    emit_debug_out(u2res[:n, j, :], j, n)
            if MAXP <= 4:
                return nc

            # ---- AllGather u2 ----
            nc.gpsimd.collective_compute(
                "AllGather", mybir.AluOpType.bypass, replica_groups=rg,
                ins=[u2d[:]], outs=[U2[:]])

            # ---- phase F: layer-2 aggregation -> out ----
            for w, n, zt in agg_layer(U2, OUT, u2res, b2bc, False, True):
                wsl = slice(w * WS, w * WS + n)
                nc.sync.dma_start(out_p[wsl, :], zt[:n, :])

    return nc


def run(cfg, inputs, sim=False, trace=False):
    from concourse.bass_utils import run_bass_kernel_spmd

    in_maps, T, b1nz, b2nz = _prepare(
        cfg, inputs["x"], inputs["edge_index"], inputs["W1"], inputs["b1"],
        inputs["W2"], inputs["b2"])
    nc = build_program(cfg, T, b1nz, b2nz)
    nc.finalize()
    core_ids = list(range(cfg.P))
    if sim:
        from concourse import bass_interp
        ms = bass_interp.MultiCoreSim(nc, cfg.P)
        for c in core_ids:
            for k, v in in_maps[c].items():
                ms.cores[c].tensor(k)[:] = v
        ms.simulate()
        outs = [np.array(ms.cores[c].tensor("out")) for c in core_ids]
        return np.concatenate(outs, axis=0), None
    res = run_bass_kernel_spmd(nc, in_maps, core_ids, trace=trace)
    outs = [np.asarray(res.results[c]["out"]) for c in core_ids]
    return np.concatenate(outs, axis=0), res


def kernel(x, edge_index, W1, b1, W2, b2):
    out, _ = run(FULL, dict(x=x, edge_index=edge_index, W1=W1, b1=b1,
                            W2=W2, b2=b2))
    return out


# revision 18
# speedup vs baseline: 2.6450x; 1.0168x over previous
"""GCN encoder (2-layer GCNConv) as a Bass/Tile kernel on 8 Trainium2 NeuronCores.

Strategy (matches the sharding hint):
  - Nodes row-partitioned across 8 cores (6250 rows each); weights replicated.
  - Symmetric normalization factorized: z = D^-1/2 (A+I) D^-1/2 (x W) + b
    =>  u = dinv * (x W);  agg[d] = u[d] + sum_{e:dst=d} u[src_e];
        z = dinv * agg + b
    so no per-edge norm gather is needed.
  - Per layer: local matmul -> row scale -> AllGather(u) -> per-core gather of
    source rows (dma_gather) -> segment-sum via tensor-engine matmuls with
    compile-time-structured 0/1 selection matrices generated on DVE
    (is_equal against an iota) -> scale/bias/relu -> output rows.
  - Edges are bucketed host-side by (dst window of 128, src half) and padded to
    128-slot tiles; padded slots gather row 0 and have an all-zero selection
    column, so they contribute nothing.  int16 gather indices require the
    src-half split (indices < 32768).
"""

import math
import os
import sys

import numpy as np

sys.path.insert(0, "/opt/trn_rl_repo")

import ml_dtypes

BF16 = ml_dtypes.bfloat16


class Cfg:
    def __init__(self, N, E, IN=512, HID=256, OUT=128, P=8, half=None):
        self.N, self.E, self.IN, self.HID, self.OUT, self.P = N, E, IN, HID, OUT, P
        self.NC = N // P                      # nodes per core
        self.WS = 128                         # dst window size
        self.NW = math.ceil(self.NC / self.WS)  # windows per core
        # src-half split point (int16 gather indices must stay < 32768)
        if half is None:
            half = N if N <= 32767 else (N + 1) // 2
        self.HALF = half
        assert self.HALF <= 32767 and N - self.HALF <= 32767


FULL = Cfg(N=50000, E=800000)


def _prepare(cfg, x, edge_index, W1, b1, W2, b2):
    """Host-side graph preprocessing -> per-core input maps + program params."""
    N, P, NC, WS, NW, HALF = cfg.N, cfg.P, cfg.NC, cfg.WS, cfg.NW, cfg.HALF
    src = np.asarray(edge_index[0], dtype=np.int64)
    dst = np.asarray(edge_index[1], dtype=np.int64)

    deg = np.bincount(dst, minlength=N).astype(np.float64) + 1.0  # + self loop
    dinv = (1.0 / np.sqrt(deg)).astype(np.float32)

    # group id: ((core, window), src-half) ; groups contiguous after sort
    win_id = (dst // NC) * NW + (dst % NC) // WS
    half = (src >= HALF).astype(np.int64)
    comp = win_id * 2 + half
    order = np.argsort(comp, kind="stable")
    s_s, d_s, c_s = src[order], dst[order], comp[order]
    counts = np.bincount(c_s, minlength=P * NW * 2).reshape(P, NW, 2)

    # shared tile counts per (window, half): max over cores
    T = np.ceil(counts.max(axis=0) / 128).astype(np.int64)  # [NW, 2]
    tiles_total = int(T.sum())
    slots_total = tiles_total * 128

    starts = np.zeros(P * NW * 2 + 1, dtype=np.int64)
    np.cumsum(counts.reshape(-1), out=starts[1:])

    dinv_pad = np.concatenate(
        [dinv, np.ones(NW * WS * P - N, dtype=np.float32)])

    in_maps = []
    for c in range(P):
        idx_arr = np.zeros(slots_total, dtype=np.int16)
        aco_arr = np.full(slots_total, -1, dtype=np.float32)  # cast to bf16 below
        off = 0
        for w in range(NW):
            for h in range(2):
                g = (c * NW + w) * 2 + h
                n = counts[c, w, h]
                sl = slice(starts[g], starts[g] + n)
                idx_arr[off:off + n] = (s_s[sl] - h * HALF).astype(np.int16)
                aco_arr[off:off + n] = (d_s[sl] - c * NC - w * WS).astype(np.float32)
                off += 128 * int(T[w, h])
        assert off == slots_total

        dloc = np.concatenate(
            [dinv[c * NC:(c + 1) * NC],
             np.ones(NW * WS - NC, dtype=np.float32)])

        m = {
            "xT": np.ascontiguousarray(
                np.asarray(x[c * NC:(c + 1) * NC], np.float32).astype(BF16).T),
            "w1": np.ascontiguousarray(
                np.asarray(W1, np.float32).astype(BF16)
                .reshape(cfg.IN // 128, 128, cfg.HID).transpose(1, 0, 2)),
            "w2": np.ascontiguousarray(
                np.asarray(W2, np.float32).astype(BF16)
                .reshape(cfg.HID // 128, 128, cfg.OUT).transpose(1, 0, 2)),
            "dinvc": np.ascontiguousarray(dloc.reshape(NW, WS).T),
            "idx": np.ascontiguousarray(np.tile(idx_arr.reshape(-1, 16).T, (8, 1))),
            "acol": np.ascontiguousarray(aco_arr.reshape(-1, 128).T.astype(BF16)),
            "ident": np.eye(128, dtype=BF16),
        }
        b1nz = bool(np.any(np.asarray(b1)))
        b2nz = bool(np.any(np.asarray(b2)))
        if b1nz:
            m["b1bc"] = np.ascontiguousarray(
                np.broadcast_to(np.asarray(b1, np.float32), (128, cfg.HID)))
        if b2nz:
            m["b2bc"] = np.ascontiguousarray(
                np.broadcast_to(np.asarray(b2, np.float32), (128, cfg.OUT)))
        in_maps.append(m)

    return in_maps, T, b1nz, b2nz


def build_program(cfg, T, b1nz, b2nz):
    import concourse.bass as bass
    import concourse.bacc as bacc
    import concourse.mybir as mybir
    from concourse import tile

    N, P, NC, WS, NW = cfg.N, cfg.P, cfg.NC, cfg.WS, cfg.NW
    IN, HID, OUT = cfg.IN, cfg.HID, cfg.OUT
    NCI, NCH = IN // 128, HID // 128
    tiles_total = int(T.sum())
    slots_total = tiles_total * 128
    f32, bf16, i16 = mybir.dt.float32, mybir.dt.bfloat16, mybir.dt.int16
    AF = mybir.ActivationFunctionType

    nc = bacc.Bacc("TRN2", target_bir_lowering=False, debug=False,
                   num_devices=cfg.P)
    xT_p = nc.dram_tensor("xT", [IN, NC], bf16, kind="ExternalInput")
    w1_p = nc.dram_tensor("w1", [128, NCI, HID], bf16, kind="ExternalInput")
    w2_p = nc.dram_tensor("w2", [128, NCH, OUT], bf16, kind="ExternalInput")
    dinv_p = nc.dram_tensor("dinvc", [WS, NW], f32, kind="ExternalInput")
    idx_p = nc.dram_tensor("idx", [128, slots_total // 16], i16, kind="ExternalInput")
    acol_p = nc.dram_tensor("acol", [128, tiles_total], bf16, kind="ExternalInput")
    id_p = nc.dram_tensor("ident", [128, 128], bf16, kind="ExternalInput")
    b1_p = (nc.dram_tensor("b1bc", [128, HID], f32, kind="ExternalInput")
            if b1nz else None)
    b2_p = (nc.dram_tensor("b2bc", [128, OUT], f32, kind="ExternalInput")
            if b2nz else None)
    out_p = nc.dram_tensor("out", [NC, OUT], f32, kind="ExternalOutput")

    u1d = nc.dram_tensor("u1d", [NC, HID], bf16)
    u2d = nc.dram_tensor("u2d", [NC, OUT], bf16)
    U1 = nc.dram_tensor("U1", [N, HID], bf16)
    U2 = nc.dram_tensor("U2", [N, OUT], bf16)
    rg = [list(range(P))]

    with tile.TileContext(nc) as tc:
        with (
            tc.tile_pool(name="res", bufs=1) as res,
            tc.tile_pool(name="work", bufs=4) as work,
            tc.tile_pool(name="gath", bufs=4) as gath,
            tc.tile_pool(name="psum", bufs=2, space="PSUM") as psum,
        ):
            # ---- resident loads ----
            xTs = res.tile([128, NCI, NC], bf16)
            for ci in range(NCI):
                nc.sync.dma_start(xTs[:, ci, :], xT_p[ci * 128:(ci + 1) * 128, :])
            w1s = res.tile([128, NCI, HID], bf16)
            nc.sync.dma_start(w1s[:], w1_p[:])
            w2s = res.tile([128, NCH, OUT], bf16)
            nc.sync.dma_start(w2s[:], w2_p[:])
            dinvs = res.tile([WS, NW], f32)
            nc.sync.dma_start(dinvs[:], dinv_p[:])
            idxs = res.tile([128, slots_total // 16], i16)
            nc.sync.dma_start(idxs[:], idx_p[:])
            acols = res.tile([128, tiles_total], bf16)
            nc.sync.dma_start(acols[:], acol_p[:])
            ident = res.tile([128, 128], bf16)
            nc.sync.dma_start(ident[:], id_p[:])
            iot = res.tile([128, 128], bf16)
            nc.gpsimd.iota(iot[:], pattern=[[1, 128]], base=0,
                           channel_multiplier=0,
                           allow_small_or_imprecise_dtypes=True)
            b1bc = None
            if b1nz:
                b1bc = res.tile([128, HID], f32)
                nc.sync.dma_start(b1bc[:], b1_p[:])
            b2bc = None
            if b2nz:
                b2bc = res.tile([128, OUT], f32)
                nc.sync.dma_start(b2bc[:], b2_p[:])

            u1res = res.tile([128, NW, HID], bf16)
            u2res = res.tile([128, NW, OUT], bf16)
            h1T = res.tile([128, NCH, NC], bf16)
            if NC % WS:
                # tail rows of the last window feed the self-loop matmul as
                # rhs; zero them so uninitialized SBUF can't inject NaNs
                nc.gpsimd.memset(u1res[:, NW - 1, :], 0.0)
                nc.gpsimd.memset(u2res[:, NW - 1, :], 0.0)

            def nsz(j):
                return min(128, NC - j * WS)

            MAXP = int(os.environ.get("GCN_MAX_PHASE", "9"))

            def emit_debug_out(src_bf16_ap, w, n):
                # convert [n, OUT] bf16 -> f32, dump into out rows of window w
                dt = work.tile([128, OUT], f32, tag="dbg")
                nc.scalar.activation(dt[:n, :], src_bf16_ap, AF.Copy)
                nc.sync.dma_start(out_p[w * WS:w * WS + n, :], dt[:n, :])

            # ---- phase A: t1 = x @ W1 ; u1 = dinv * t1 ----
            for j in range(NW):
                n = nsz(j)
                jsl = slice(j * WS, j * WS + n)
                pt = psum.tile([128, HID], f32, tag="mm")
                for ci in range(NCI):
                    nc.tensor.matmul(pt[:n, :], xTs[:, ci, jsl],
                                     w1s[:, ci, :], start=(ci == 0),
                                     stop=(ci == NCI - 1))
                nc.scalar.activation(u1res[:n, j, :], pt[:n, :], AF.Copy,
                                     scale=dinvs[:n, j:j + 1])
                nc.sync.dma_start(u1d[jsl, :], u1res[:n, j, :])
                if MAXP == 1:
                    emit_debug_out(u1res[:n, j, :OUT], j, n)
            if MAXP <= 1:
                return nc

            # ---- AllGather u1 ----
            nc.gpsimd.collective_compute(
                "AllGather", mybir.AluOpType.bypass, replica_groups=rg,
                ins=[u1d[:]], outs=[U1[:]])
            if MAXP == 2:
                for j in range(NW):
                    n = nsz(j)
                    gt = work.tile([128, OUT], bf16, tag="dbg_g")
                    nc.sync.dma_start(gt[:n, :], U1[j * WS:j * WS + n, :OUT])
                    emit_debug_out(gt[:n, :], j, n)
                return nc

            # ---- generic aggregation layer ----
            def agg_layer(U, F, ures, bbc, relu, emit_out):
                tile_idx = 0
                slot_off = 0
                for w in range(NW):
                    n = nsz(w)
                    pa = psum.tile([128, F], f32, tag="agg")
                    # self-loop term: ident.T @ u[w]
                    nc.tensor.matmul(pa[:n, :], ident[:, :n], ures[:, w, :],
                                     start=True, stop=False)
                    nmm = int(T[w, 0] + T[w, 1])
                    done = 0
                    for h in range(2):
                        t_wh = int(T[w, h])
                        if t_wh == 0:
                            continue
                        g = gath.tile([128, t_wh, F], bf16, tag="g")
                        base = 0 if h == 0 else cfg.HALF
                        nc.gpsimd.dma_gather(
                            g[:], U[base:base + min(cfg.HALF, N - base), :],
                            idxs[:, slot_off // 16:
                                 (slot_off + 128 * t_wh) // 16],
                            num_idxs=128 * t_wh, num_idxs_reg=128 * t_wh,
                            elem_size=F, single_packet=False)
                        slot_off += 128 * t_wh
                        for t in range(t_wh):
                            S = work.tile([128, 128], bf16, tag="S")
                            nc.vector.tensor_tensor(
                                S[:], iot[:],
                                acols[:, tile_idx:tile_idx + 1]
                                .broadcast_to((128, 128)),
                                op=mybir.AluOpType.is_equal)
                            tile_idx += 1
                            done += 1
                            nc.tensor.matmul(pa[:n, :], S[:, :n], g[:, t, :],
                                             start=False, stop=(done == nmm))
                    # z = dinv * agg (+ b) ; relu
                    if bbc is None:
                        zf = AF.Relu if relu else AF.Copy
                        zt = work.tile([128, F], f32 if emit_out else bf16,
                                       tag="zt%d" % F)
                        nc.scalar.activation(zt[:n, :], pa[:n, :], zf,
                                             scale=dinvs[:n, w:w + 1])
                    else:
                        v = work.tile([128, F], f32, tag="v%d" % F)
                        nc.scalar.activation(v[:n, :], pa[:n, :], AF.Copy,
                                             scale=dinvs[:n, w:w + 1])
                        zt = work.tile([128, F], f32 if emit_out else bf16,
                                       tag="zt%d" % F)
                        if relu:
                            vb = work.tile([128, F], f32, tag="vb%d" % F)
                            nc.vector.tensor_tensor(
                                vb[:n, :], v[:n, :], bbc[:n, :],
                                op=mybir.AluOpType.add)
                            nc.scalar.activation(zt[:n, :], vb[:n, :], AF.Relu)
                        else:
                            nc.vector.tensor_tensor(
                                zt[:n, :], v[:n, :], bbc[:n, :],
                                op=mybir.AluOpType.add)
                    yield w, n, zt

            # ---- phase C: layer-1 aggregation -> h1 -> h1T ----
            for w, n, zt in agg_layer(U1, HID, u1res, b1bc, True, False):
                wsl = slice(w * WS, w * WS + n)
                for ch in range(NCH):
                    ptr = psum.tile([128, 128], bf16, tag="tr")
                    nc.tensor.transpose(ptr[:, :n],
                                        zt[:n, ch * 128:(ch + 1) * 128],
                                        ident[:n, :n])
                    nc.scalar.activation(h1T[:, ch, wsl], ptr[:, :n], AF.Copy)
                if MAXP == 3:
                    emit_debug_out(zt[:n, :OUT], w, n)
            if MAXP <= 3:
                return nc

            # ---- phase D: t2 = h1 @ W2 ; u2 ----
            for j in range(NW):
                n = nsz(j)
                jsl = slice(j * WS, j * WS + n)
                pt = psum.tile([128, OUT], f32, tag="mm")
                for ch in range(NCH):
                    nc.tensor.matmul(pt[:n, :], h1T[:, ch, jsl],
                                     w2s[:, ch, :], start=(ch == 0),
                                     stop=(ch == NCH - 1))
                nc.scalar.activation(u2res[:n, j, :], pt[:n, :], AF.Copy,
                                     scale=dinvs[:n, j:j + 1])
                nc.sync.dma_start(u2d[jsl, :], u2res[:n, j, :])
                if MAXP == 4:
                    emit_debug_out(u2res[:n, j, :], j, n)
            if MAXP <= 4:
                return nc

            # ---- AllGather u2 ----
            nc.gpsimd.collective_compute(
                "AllGather", mybir.AluOpType.bypass, replica_groups=rg,
                ins=[u2d[:]], outs=[U2[:]])

            # ---- phase F: layer-2 aggregation -> out ----
            for w, n, zt in agg_layer(U2, OUT, u2res, b2bc, False, True):
                wsl = slice(w * WS, w * WS + n)
                nc.sync.dma_start(out_p[wsl, :], zt[:n, :])

    return nc


def run(cfg, inputs, sim=False, trace=False):
    from concourse.bass_utils import run_bass_kernel_spmd

    in_maps, T, b1nz, b2nz = _prepare(
        cfg, inputs["x"], inputs["edge_index"], inputs["W1"], inputs["b1"],
        inputs["W2"], inputs["b2"])
    nc = build_program(cfg, T, b1nz, b2nz)
    nc.finalize()
    core_ids = list(range(cfg.P))
    if sim:
        from concourse import bass_interp
        ms = bass_interp.MultiCoreSim(nc, cfg.P)
        for c in core_ids:
            for k, v in in_maps[c].items():
                ms.cores[c].tensor(k)[:] = v
        ms.simulate()
        outs = [np.array(ms.cores[c].tensor("out")) for c in core_ids]
        return np.concatenate(outs, axis=0), None
    res = run_bass_kernel_spmd(nc, in_maps, core_ids, trace=trace)
    outs = [np.asarray(res.results[c]["out"]) for c in core_ids]
    return np.concatenate(outs, axis=0), res


def kernel(x, edge_index, W1, b1, W2, b2):
    out, _ = run(FULL, dict(x=x, edge_index=edge_index, W1=W1, b1=b1,
                            W2=W2, b2=b2))
    return out
